# revision 21
# baseline (speedup 1.0000x reference)
"""Trainium2 Bass kernel for C3ALinear: y = x @ W.T + b + block_circconv(x, k)/D.

The block-circular convolution is algebraically a matmul with a block-circulant
matrix, so the whole op folds into a single matmul with
W_comb = base_weight + circulant_expand(c3a_kernel)/D_IN.  The 8192x4096x4096
matmul runs data-parallel over tokens on 8 NeuronCores (1024 tokens/core) with
float32r (full-rate fp32) PE matmuls.
"""
import sys

sys.path.insert(0, "/opt/trn_rl_repo")

import numpy as np

B, S, D_IN, D_OUT, BLK = 4, 2048, 4096, 4096, 256
N_CORES = 8
TOK = B * S              # 8192 tokens
TOK_SH = TOK // N_CORES  # 1024 tokens per core
P = 128                  # partitions
NF = 512                 # matmul free dim (one PSUM bank of fp32)
KT = D_IN // P           # 32 contraction tiles
MT = TOK_SH // P         # 8 token tiles per core
NT = D_OUT // NF         # 8 output-feature panels

_cache = {}


def _build_nc(repeats=None):
    import contextlib

    import concourse.mybir as mybir
    import concourse.tile as tile
    from concourse import bacc

    nc = bacc.Bacc(None, target_bir_lowering=False, debug=False)

    xT = nc.dram_tensor("xT", [D_IN, TOK_SH], mybir.dt.float32, kind="ExternalInput")
    wT = nc.dram_tensor("wT", [D_IN, D_OUT], mybir.dt.float32, kind="ExternalInput")
    biasb = nc.dram_tensor("biasb", [P, D_OUT], mybir.dt.float32, kind="ExternalInput")
    out = nc.dram_tensor("out", [TOK_SH, D_OUT], mybir.dt.float32, kind="ExternalOutput")

    with tile.TileContext(nc) as tc:
        with tc.tile_pool(name="xs", bufs=2) as xs_pool, \
             tc.tile_pool(name="ws", bufs=6) as ws_pool, \
             tc.tile_pool(name="wr", bufs=10) as wr_pool, \
             tc.tile_pool(name="bias", bufs=2) as bias_pool, \
             tc.tile_pool(name="ob", bufs=6) as ob_pool, \
             tc.tile_pool(name="xr", bufs=KT) as xr_pool, \
             tc.tile_pool(name="ps", bufs=8, space="PSUM") as ps_pool:

            if repeats is not None:
                loop_cm = tc.For_i(
                    0, repeats, 1,
                    hint_engines=(
                        mybir.EngineType.PE, mybir.EngineType.DVE,
                        mybir.EngineType.Activation, mybir.EngineType.SP,
                        mybir.EngineType.Pool,
                    ),
                )
            else:
                loop_cm = contextlib.nullcontext()

            with loop_cm:
                # x shard loads are interleaved into panel 0's k-loop so the
                # first W tile isn't queued behind 16 MB of x DMA.
                x_r = [None] * KT

                for n in range(NT):
                    bias_t = bias_pool.tile([P, NF], mybir.dt.float32, tag="bias")
                    nc.sync.dma_start(bias_t[:], biasb[:, n * NF:(n + 1) * NF])
                    psums = [
                        ps_pool.tile([P, NF], mybir.dt.float32, tag="ps", name="ps")
                        for _ in range(MT)
                    ]
                    for k in range(KT):
                        if n == 0:
                            xs = xs_pool.tile([P, TOK_SH], mybir.dt.float32, tag="xs")
                            nc.sync.dma_start(xs[:], xT[k * P:(k + 1) * P, :])
                            xr = xr_pool.tile([P, TOK_SH], mybir.dt.float32r, tag="xr")
                            nc.vector.tensor_copy(xr[:], xs[:])
                            x_r[k] = xr
                        ws = ws_pool.tile([P, NF], mybir.dt.float32, tag="ws")
                        nc.sync.dma_start(
                            ws[:], wT[k * P:(k + 1) * P, n * NF:(n + 1) * NF])
                        wr = wr_pool.tile([P, NF], mybir.dt.float32r, tag="wr")
                        nc.vector.tensor_copy(wr[:], ws[:])
                        for m in range(MT):
                            nc.tensor.matmul(
                                psums[m][:],
                                x_r[k][:, m * P:(m + 1) * P],
                                wr[:],
                                start=(k == 0),
                                stop=(k == KT - 1),
                            )
                    for m in range(MT):
                        ob = ob_pool.tile([P, NF], mybir.dt.float32, tag="ob")
                        nc.any.tensor_add(ob[:], psums[m][:], bias_t[:])
                        nc.sync.dma_start(
                            out[m * P:(m + 1) * P, n * NF:(n + 1) * NF], ob[:]
                        )

    nc.compile()
    return nc


def _build_nc_v2(repeats=None):
    """xstat with in-place f32r rounding (no f32 staging pools: DMA lands in
    the f32r tile via a bitcast view, DVE rounds in place), W DMAs on the
    sync HWDGE ring, x/out DMAs on the scalar HWDGE ring, wider wr pool."""
    import contextlib

    import concourse.mybir as mybir
    import concourse.tile as tile
    from concourse import bacc

    nc = bacc.Bacc(None, target_bir_lowering=False, debug=False)

    f32 = mybir.dt.float32
    f32r = mybir.dt.float32r
    xT = nc.dram_tensor("xT", [D_IN, TOK_SH], f32, kind="ExternalInput")
    wT = nc.dram_tensor("wT", [D_IN, D_OUT], f32, kind="ExternalInput")
    biasb = nc.dram_tensor("biasb", [P, D_OUT], f32, kind="ExternalInput")
    out = nc.dram_tensor("out", [TOK_SH, D_OUT], f32, kind="ExternalOutput")

    with tile.TileContext(nc) as tc:
        with tc.tile_pool(name="wr", bufs=16) as wr_pool, \
             tc.tile_pool(name="bias", bufs=2) as bias_pool, \
             tc.tile_pool(name="ob", bufs=8) as ob_pool, \
             tc.tile_pool(name="xr", bufs=KT) as xr_pool, \
             tc.tile_pool(name="ps", bufs=8, space="PSUM") as ps_pool:

            if repeats is not None:
                loop_cm = tc.For_i(
                    0, repeats, 1,
                    hint_engines=(
                        mybir.EngineType.PE, mybir.EngineType.DVE,
                        mybir.EngineType.Activation, mybir.EngineType.SP,
                        mybir.EngineType.Pool,
                    ),
                )
            else:
                loop_cm = contextlib.nullcontext()

            with loop_cm:
                x_r = [None] * KT

                for n in range(NT):
                    bias_t = bias_pool.tile([P, NF], f32, tag="bias")
                    nc.sync.dma_start(bias_t[:], biasb[:, n * NF:(n + 1) * NF])
                    psums = [
                        ps_pool.tile([P, NF], f32, tag="ps", name="ps")
                        for _ in range(MT)
                    ]
                    for k in range(KT):
                        if n == 0:
                            xr = xr_pool.tile([P, TOK_SH], f32r, tag="xr")
                            nc.scalar.dma_start(
                                xr[:].bitcast(f32), xT[k * P:(k + 1) * P, :])
                            nc.vector.tensor_copy(xr[:], xr[:].bitcast(f32))
                            x_r[k] = xr
                        wr = wr_pool.tile([P, NF], f32r, tag="wr")
                        nc.sync.dma_start(
                            wr[:].bitcast(f32),
                            wT[k * P:(k + 1) * P, n * NF:(n + 1) * NF])
                        nc.vector.tensor_copy(wr[:], wr[:].bitcast(f32))
                        for m in range(MT):
                            nc.tensor.matmul(
                                psums[m][:],
                                x_r[k][:, m * P:(m + 1) * P],
                                wr[:],
                                start=(k == 0),
                                stop=(k == KT - 1),
                            )
                    for m in range(MT):
                        ob = ob_pool.tile([P, NF], f32, tag="ob")
                        nc.any.tensor_add(ob[:], psums[m][:], bias_t[:])
                        nc.scalar.dma_start(
                            out[m * P:(m + 1) * P, n * NF:(n + 1) * NF], ob[:]
                        )

    nc.compile()
    return nc


def _build_nc_wstat(repeats=None):
    """W-stationary layout: out.T[d_out, tok] per core; lhsT = W tile reused
    across 2 moving token-blocks (halves exposed f32r weight-load cost);
    bias is per-partition via tensor_scalar_add."""
    import contextlib

    import concourse.mybir as mybir
    import concourse.tile as tile
    from concourse import bacc

    DG = 8            # d_out groups of NF=512 (4 d-tiles of 128)
    TB = TOK_SH // NF  # 2 token blocks

    nc = bacc.Bacc(None, target_bir_lowering=False, debug=False)

    xT = nc.dram_tensor("xT", [D_IN, TOK_SH], mybir.dt.float32, kind="ExternalInput")
    wT = nc.dram_tensor("wT", [D_IN, D_OUT], mybir.dt.float32, kind="ExternalInput")
    bias_col = nc.dram_tensor(
        "bias_col", [P, D_OUT // P], mybir.dt.float32, kind="ExternalInput")
    outT = nc.dram_tensor(
        "outT", [D_OUT, TOK_SH], mybir.dt.float32, kind="ExternalOutput")

    with tile.TileContext(nc) as tc:
        with tc.tile_pool(name="xs", bufs=2) as xs_pool, \
             tc.tile_pool(name="xr", bufs=KT) as xr_pool, \
             tc.tile_pool(name="ws", bufs=4) as ws_pool, \
             tc.tile_pool(name="wr", bufs=6) as wr_pool, \
             tc.tile_pool(name="bias", bufs=1) as bias_pool, \
             tc.tile_pool(name="ob", bufs=4) as ob_pool, \
             tc.tile_pool(name="ps", bufs=8, space="PSUM") as ps_pool:

            if repeats is not None:
                loop_cm = tc.For_i(
                    0, repeats, 1,
                    hint_engines=(
                        mybir.EngineType.PE, mybir.EngineType.DVE,
                        mybir.EngineType.Activation, mybir.EngineType.SP,
                        mybir.EngineType.Pool,
                    ),
                )
            else:
                loop_cm = contextlib.nullcontext()

            with loop_cm:
                bias_t = bias_pool.tile([P, D_OUT // P], mybir.dt.float32, tag="bias")
                nc.sync.dma_start(bias_t[:], bias_col[:])

                x_r = [None] * KT

                for dg in range(DG):
                    psums = [
                        ps_pool.tile([P, NF], mybir.dt.float32, tag="ps", name="ps")
                        for _ in range(4 * TB)
                    ]
                    for k in range(KT):
                        if dg == 0:
                            xs = xs_pool.tile([P, TOK_SH], mybir.dt.float32, tag="xs")
                            nc.sync.dma_start(xs[:], xT[k * P:(k + 1) * P, :])
                            xr = xr_pool.tile([P, TOK_SH], mybir.dt.float32r, tag="xr")
                            nc.vector.tensor_copy(xr[:], xs[:])
                            x_r[k] = xr
                        ws = ws_pool.tile([P, NF], mybir.dt.float32, tag="ws")
                        nc.sync.dma_start(
                            ws[:], wT[k * P:(k + 1) * P, dg * NF:(dg + 1) * NF])
                        wr = wr_pool.tile([P, NF], mybir.dt.float32r, tag="wr")
                        nc.vector.tensor_copy(wr[:], ws[:])
                        for j in range(4):
                            for t in range(TB):
                                nc.tensor.matmul(
                                    psums[j * TB + t][:],
                                    wr[:, j * P:(j + 1) * P],
                                    x_r[k][:, t * NF:(t + 1) * NF],
                                    start=(k == 0),
                                    stop=(k == KT - 1),
                                )
                    for j in range(4):
                        d = dg * 4 + j
                        for t in range(TB):
                            ob = ob_pool.tile([P, NF], mybir.dt.float32, tag="ob")
                            nc.vector.tensor_scalar_add(
                                ob[:], psums[j * TB + t][:], bias_t[:, d:d + 1])
                            nc.sync.dma_start(
                                outT[d * P:(d + 1) * P, t * NF:(t + 1) * NF], ob[:])

    nc.compile()
    return nc


def _build_nc_wstat2(repeats=None):
    """W-stationary with half-group PSUM alternation: each d-group of 512
    outputs is processed as two halves of 4 PSUM tiles alternating between
    bank groups 0-3 and 4-7, so evictions of one half overlap compute of the
    next and the PE never waits on PSUM recycling.  W is read once as
    [128,256] half-tiles; x stays resident in f32r."""
    import contextlib

    import concourse.mybir as mybir
    import concourse.tile as tile
    from concourse import bacc

    DG = 8             # d_out groups of NF=512
    TB = TOK_SH // NF  # 2 token blocks
    HNF = NF // 2      # 256: W half-tile width

    nc = bacc.Bacc(None, target_bir_lowering=False, debug=False)

    xT = nc.dram_tensor("xT", [D_IN, TOK_SH], mybir.dt.float32, kind="ExternalInput")
    wT = nc.dram_tensor("wT", [D_IN, D_OUT], mybir.dt.float32, kind="ExternalInput")
    bias_col = nc.dram_tensor(
        "bias_col", [P, D_OUT // P], mybir.dt.float32, kind="ExternalInput")
    outT = nc.dram_tensor(
        "outT", [D_OUT, TOK_SH], mybir.dt.float32, kind="ExternalOutput")

    with tile.TileContext(nc) as tc:
        with tc.tile_pool(name="xs", bufs=2) as xs_pool, \
             tc.tile_pool(name="xr", bufs=KT) as xr_pool, \
             tc.tile_pool(name="ws", bufs=6) as ws_pool, \
             tc.tile_pool(name="wr", bufs=8) as wr_pool, \
             tc.tile_pool(name="bias", bufs=1) as bias_pool, \
             tc.tile_pool(name="ob", bufs=6) as ob_pool, \
             tc.tile_pool(name="ps", bufs=8, space="PSUM") as ps_pool:

            if repeats is not None:
                loop_cm = tc.For_i(
                    0, repeats, 1,
                    hint_engines=(
                        mybir.EngineType.PE, mybir.EngineType.DVE,
                        mybir.EngineType.Activation, mybir.EngineType.SP,
                        mybir.EngineType.Pool,
                    ),
                )
            else:
                loop_cm = contextlib.nullcontext()

            with loop_cm:
                bias_t = bias_pool.tile([P, D_OUT // P], mybir.dt.float32, tag="bias")
                nc.sync.dma_start(bias_t[:], bias_col[:])

                x_r = [None] * KT

                def evict(dg, h, psums):
                    # evictions of half (dg,h): d tiles dg*4+2h, dg*4+2h+1
                    evs = []
                    for j in range(2):
                        d = dg * 4 + 2 * h + j
                        for t in range(TB):
                            evs.append((d, t, psums[j * TB + t]))
                    return evs

                def emit_evict(ev):
                    d, t, psum = ev
                    ob = ob_pool.tile([P, NF], mybir.dt.float32, tag="ob", name="ob")
                    nc.vector.tensor_scalar_add(ob[:], psum[:], bias_t[:, d:d + 1])
                    nc.sync.dma_start(
                        outT[d * P:(d + 1) * P, t * NF:(t + 1) * NF], ob[:])

                pending = []
                for dg in range(DG):
                    for h in range(2):
                        psums = [
                            ps_pool.tile([P, NF], mybir.dt.float32, tag="ps",
                                         name="ps")
                            for _ in range(4)
                        ]
                        for k in range(KT):
                            if dg == 0 and h == 0:
                                xs = xs_pool.tile(
                                    [P, TOK_SH], mybir.dt.float32, tag="xs")
                                nc.sync.dma_start(xs[:], xT[k * P:(k + 1) * P, :])
                                xr = xr_pool.tile(
                                    [P, TOK_SH], mybir.dt.float32r, tag="xr")
                                nc.vector.tensor_copy(xr[:], xs[:])
                                x_r[k] = xr
                            ws = ws_pool.tile([P, HNF], mybir.dt.float32, tag="ws")
                            nc.sync.dma_start(
                                ws[:],
                                wT[k * P:(k + 1) * P,
                                   dg * NF + h * HNF:dg * NF + (h + 1) * HNF])
                            wr = wr_pool.tile([P, HNF], mybir.dt.float32r, tag="wr")
                            nc.vector.tensor_copy(wr[:], ws[:])
                            if pending and k < len(pending):
                                emit_evict(pending[k])
                            for j in range(2):
                                for t in range(TB):
                                    nc.tensor.matmul(
                                        psums[j * TB + t][:],
                                        wr[:, j * P:(j + 1) * P],
                                        x_r[k][:, t * NF:(t + 1) * NF],
                                        start=(k == 0),
                                        stop=(k == KT - 1),
                                    )
                        pending = evict(dg, h, psums)
                for ev in pending:
                    emit_evict(ev)

    nc.compile()
    return nc


def make_runner(nc, n_cores=N_CORES, replicated_inputs=()):
    """Build a reusable jitted SPMD callable for a compiled Bass module.

    Mirrors bass2jax.run_bass_via_pjrt's multi-core path, but returns the
    jitted function so repeated calls don't re-trace/re-compile.
    Inputs named in `replicated_inputs` use a replicated spec (pass the
    plain per-core array, no 8x concat)."""
    import jax
    import concourse.mybir as mybir
    from concourse import bass2jax
    from jax.experimental.shard_map import shard_map
    from jax.sharding import Mesh, PartitionSpec

    bass2jax.install_neuronx_cc_hook()

    partition_name = nc.partition_id_tensor.name if nc.partition_id_tensor else None
    in_names, out_names, out_avals, zero_outs = [], [], [], []
    for alloc in nc.m.functions[0].allocations:
        if not isinstance(alloc, mybir.MemoryLocationSet):
            continue
        name = alloc.memorylocations[0].name
        if alloc.kind == "ExternalInput":
            if name != partition_name:
                in_names.append(name)
        elif alloc.kind == "ExternalOutput":
            out_names.append(name)
            shape = tuple(alloc.tensor_shape)
            dtype = mybir.dt.np(alloc.dtype)
            out_avals.append(jax.core.ShapedArray(shape, dtype))
            zero_outs.append(np.zeros(shape, dtype))
    n_params = len(in_names)
    n_outs = len(out_avals)
    bind_in_names = list(in_names) + list(out_names)
    if partition_name is not None:
        bind_in_names.append(partition_name)

    def _body(*args):
        operands = list(args)
        if partition_name is not None:
            operands.append(bass2jax.partition_id_tensor())
        outs = bass2jax._bass_exec_p.bind(
            *operands,
            out_avals=tuple(out_avals),
            in_names=tuple(bind_in_names),
            out_names=tuple(out_names),
            lowering_input_output_aliases=(),
            sim_require_finite=True,
            sim_require_nnan=True,
            nc=nc,
        )
        return tuple(outs)

    devices = jax.devices()[:n_cores]
    mesh = Mesh(np.asarray(devices), ("core",))
    specs_map = {
        name: (PartitionSpec() if name in replicated_inputs
               else PartitionSpec("core"))
        for name in in_names
    }
    in_specs = tuple(specs_map[name] for name in in_names) + \
        (PartitionSpec("core"),) * n_outs
    out_specs = (PartitionSpec("core"),) * n_outs
    donate = tuple(range(n_params, n_params + n_outs))
    fn = jax.jit(
        shard_map(_body, mesh=mesh, in_specs=in_specs, out_specs=out_specs,
                  check_rep=False),
        donate_argnums=donate,
        keep_unused=True,
    )
    return {
        "fn": fn,
        "body": _body,
        "n_params": n_params,
        "in_names": in_names,
        "in_specs_map": specs_map,
        "out_names": out_names,
        "out_avals": out_avals,
        "zero_outs": zero_outs,
        "mesh": mesh,
        "n_cores": n_cores,
    }


LAYOUT = "xstat"  # "xstat" (out=[tok,d_out]) or "wstat" (out=[d_out,tok])


def build_nc(layout=None, repeats=None):
    layout = layout or LAYOUT
    builders = {"xstat": _build_nc, "xstat2": _build_nc_v2,
                "wstat": _build_nc_wstat, "wstat2": _build_nc_wstat2}
    return builders[layout](repeats=repeats)


def _get_runner(layout=None):
    layout = layout or LAYOUT
    key = f"runner_{layout}"
    if key not in _cache:
        repl = (("wT", "biasb") if layout in ("xstat", "xstat2")
                else ("wT", "bias_col"))
        _cache[key] = make_runner(build_nc(layout=layout), replicated_inputs=repl)
    return _cache[key]


def _circulant_expand(kernel):
    # W[p*b+i, q*b+j] = kernel[p, q, (i-j) % b]
    p, q, b = kernel.shape
    idx = (np.arange(b)[:, None] - np.arange(b)[None, :]) % b
    kc = kernel[:, :, idx]  # (p, q, b_i, b_j)
    return kc.transpose(0, 2, 1, 3).reshape(p * b, q * b)


def prep_inputs(x, base_weight, base_bias, c3a_kernel, layout=None):
    """Host-side prep: fold c3a into the weight; emit per-input concat arrays
    (axis 0 concat over cores, as shard_map expects)."""
    layout = layout or LAYOUT
    x = np.asarray(x, dtype=np.float32)
    base_weight = np.asarray(base_weight, dtype=np.float32)
    base_bias = np.asarray(base_bias, dtype=np.float32)
    c3a_kernel = np.asarray(c3a_kernel, dtype=np.float32)

    w_comb = base_weight + _circulant_expand(c3a_kernel) * (1.0 / D_IN)
    wT = np.ascontiguousarray(w_comb.T)                      # [D_IN, D_OUT]
    xT = np.ascontiguousarray(x.reshape(TOK, D_IN).T)        # [D_IN, TOK]

    # per-core shards, concatenated along axis 0 (shard_map splits axis 0)
    xT_cat = np.concatenate(
        [xT[:, c * TOK_SH:(c + 1) * TOK_SH] for c in range(N_CORES)], axis=0)
    if layout == "xstat":
        biasb = np.ascontiguousarray(
            np.broadcast_to(base_bias, (P, D_OUT)).astype(np.float32))
        return {"xT": xT_cat, "wT": wT, "biasb": biasb}
    else:
        bias_col = np.ascontiguousarray(base_bias.reshape(D_OUT // P, P).T)
        return {"xT": xT_cat, "wT": wT, "bias_col": bias_col}


def assemble_output(out_global, layout=None):
    """out_global: the concat-over-cores output array -> full (B,S,D_OUT)."""
    layout = layout or LAYOUT
    if layout == "xstat":
        # (N_CORES*TOK_SH, D_OUT), token-sharded in order
        return np.asarray(out_global).reshape(B, S, D_OUT)
    else:
        # (N_CORES*D_OUT, TOK_SH) -> [c, d, t] -> full [d, c*t]
        a = np.asarray(out_global).reshape(N_CORES, D_OUT, TOK_SH)
        full = a.transpose(1, 0, 2).reshape(D_OUT, TOK)
        return np.ascontiguousarray(full.T).reshape(B, S, D_OUT)


def kernel(x, base_weight, base_bias, c3a_kernel, **_):
    runner = _get_runner()
    cat = prep_inputs(x, base_weight, base_bias, c3a_kernel)
    ins = [cat[name] for name in runner["in_names"]]
    zeros = [
        np.zeros((N_CORES * z.shape[0], *z.shape[1:]), z.dtype)
        for z in runner["zero_outs"]
    ]
    out_arrs = runner["fn"](*ins, *zeros)
    return assemble_output(out_arrs[0])


# revision 23
# speedup vs baseline: 1.0169x; 1.0169x over previous
"""Trainium2 Bass kernel for C3ALinear: y = x @ W.T + b + block_circconv(x, k)/D.

The block-circular convolution is algebraically a matmul with a block-circulant
matrix, so the whole op folds into a single matmul with
W_comb = base_weight + circulant_expand(c3a_kernel)/D_IN.  The 8192x4096x4096
matmul runs data-parallel over tokens on 8 NeuronCores (1024 tokens/core) with
float32r (full-rate fp32) PE matmuls.
"""
import sys

sys.path.insert(0, "/opt/trn_rl_repo")

import numpy as np

B, S, D_IN, D_OUT, BLK = 4, 2048, 4096, 4096, 256
N_CORES = 8
TOK = B * S              # 8192 tokens
TOK_SH = TOK // N_CORES  # 1024 tokens per core
P = 128                  # partitions
NF = 512                 # matmul free dim (one PSUM bank of fp32)
KT = D_IN // P           # 32 contraction tiles
MT = TOK_SH // P         # 8 token tiles per core
NT = D_OUT // NF         # 8 output-feature panels

_cache = {}


def _build_nc(repeats=None, evict="any", wr_bufs=10, ws_bufs=6, out_ring="sync"):
    import contextlib

    import concourse.mybir as mybir
    import concourse.tile as tile
    from concourse import bacc

    nc = bacc.Bacc(None, target_bir_lowering=False, debug=False)

    xT = nc.dram_tensor("xT", [D_IN, TOK_SH], mybir.dt.float32, kind="ExternalInput")
    wT = nc.dram_tensor("wT", [D_IN, D_OUT], mybir.dt.float32, kind="ExternalInput")
    biasb = nc.dram_tensor("biasb", [P, D_OUT], mybir.dt.float32, kind="ExternalInput")
    out = nc.dram_tensor("out", [TOK_SH, D_OUT], mybir.dt.float32, kind="ExternalOutput")

    evict_eng = {"any": nc.any, "vector": nc.vector}[evict]
    out_eng = {"sync": nc.sync, "scalar": nc.scalar}[out_ring]

    with tile.TileContext(nc) as tc:
        with tc.tile_pool(name="xs", bufs=2) as xs_pool, \
             tc.tile_pool(name="ws", bufs=ws_bufs) as ws_pool, \
             tc.tile_pool(name="wr", bufs=wr_bufs) as wr_pool, \
             tc.tile_pool(name="bias", bufs=2) as bias_pool, \
             tc.tile_pool(name="ob", bufs=6) as ob_pool, \
             tc.tile_pool(name="xr", bufs=KT) as xr_pool, \
             tc.tile_pool(name="ps", bufs=8, space="PSUM") as ps_pool:

            if repeats is not None:
                loop_cm = tc.For_i(
                    0, repeats, 1,
                    hint_engines=(
                        mybir.EngineType.PE, mybir.EngineType.DVE,
                        mybir.EngineType.Activation, mybir.EngineType.SP,
                        mybir.EngineType.Pool,
                    ),
                )
            else:
                loop_cm = contextlib.nullcontext()

            with loop_cm:
                # x shard loads are interleaved into panel 0's k-loop so the
                # first W tile isn't queued behind 16 MB of x DMA.
                x_r = [None] * KT

                for n in range(NT):
                    bias_t = bias_pool.tile([P, NF], mybir.dt.float32, tag="bias")
                    nc.sync.dma_start(bias_t[:], biasb[:, n * NF:(n + 1) * NF])
                    psums = [
                        ps_pool.tile([P, NF], mybir.dt.float32, tag="ps", name="ps")
                        for _ in range(MT)
                    ]
                    for k in range(KT):
                        if n == 0:
                            xs = xs_pool.tile([P, TOK_SH], mybir.dt.float32, tag="xs")
                            nc.sync.dma_start(xs[:], xT[k * P:(k + 1) * P, :])
                            xr = xr_pool.tile([P, TOK_SH], mybir.dt.float32r, tag="xr")
                            nc.vector.tensor_copy(xr[:], xs[:])
                            x_r[k] = xr
                        ws = ws_pool.tile([P, NF], mybir.dt.float32, tag="ws")
                        nc.sync.dma_start(
                            ws[:], wT[k * P:(k + 1) * P, n * NF:(n + 1) * NF])
                        wr = wr_pool.tile([P, NF], mybir.dt.float32r, tag="wr")
                        nc.vector.tensor_copy(wr[:], ws[:])
                        for m in range(MT):
                            nc.tensor.matmul(
                                psums[m][:],
                                x_r[k][:, m * P:(m + 1) * P],
                                wr[:],
                                start=(k == 0),
                                stop=(k == KT - 1),
                            )
                    for m in range(MT):
                        ob = ob_pool.tile([P, NF], mybir.dt.float32, tag="ob")
                        evict_eng.tensor_add(ob[:], psums[m][:], bias_t[:])
                        out_eng.dma_start(
                            out[m * P:(m + 1) * P, n * NF:(n + 1) * NF], ob[:]
                        )

    nc.compile()
    return nc


def _build_nc_v2(repeats=None):
    """xstat with in-place f32r rounding (no f32 staging pools: DMA lands in
    the f32r tile via a bitcast view, DVE rounds in place), W DMAs on the
    sync HWDGE ring, x/out DMAs on the scalar HWDGE ring, wider wr pool."""
    import contextlib

    import concourse.mybir as mybir
    import concourse.tile as tile
    from concourse import bacc

    nc = bacc.Bacc(None, target_bir_lowering=False, debug=False)

    f32 = mybir.dt.float32
    f32r = mybir.dt.float32r
    xT = nc.dram_tensor("xT", [D_IN, TOK_SH], f32, kind="ExternalInput")
    wT = nc.dram_tensor("wT", [D_IN, D_OUT], f32, kind="ExternalInput")
    biasb = nc.dram_tensor("biasb", [P, D_OUT], f32, kind="ExternalInput")
    out = nc.dram_tensor("out", [TOK_SH, D_OUT], f32, kind="ExternalOutput")

    with tile.TileContext(nc) as tc:
        with tc.tile_pool(name="wr", bufs=16) as wr_pool, \
             tc.tile_pool(name="bias", bufs=2) as bias_pool, \
             tc.tile_pool(name="ob", bufs=8) as ob_pool, \
             tc.tile_pool(name="xr", bufs=KT) as xr_pool, \
             tc.tile_pool(name="ps", bufs=8, space="PSUM") as ps_pool:

            if repeats is not None:
                loop_cm = tc.For_i(
                    0, repeats, 1,
                    hint_engines=(
                        mybir.EngineType.PE, mybir.EngineType.DVE,
                        mybir.EngineType.Activation, mybir.EngineType.SP,
                        mybir.EngineType.Pool,
                    ),
                )
            else:
                loop_cm = contextlib.nullcontext()

            with loop_cm:
                x_r = [None] * KT

                for n in range(NT):
                    bias_t = bias_pool.tile([P, NF], f32, tag="bias")
                    nc.sync.dma_start(bias_t[:], biasb[:, n * NF:(n + 1) * NF])
                    psums = [
                        ps_pool.tile([P, NF], f32, tag="ps", name="ps")
                        for _ in range(MT)
                    ]
                    for k in range(KT):
                        if n == 0:
                            xr = xr_pool.tile([P, TOK_SH], f32r, tag="xr")
                            nc.scalar.dma_start(
                                xr[:].bitcast(f32), xT[k * P:(k + 1) * P, :])
                            nc.vector.tensor_copy(xr[:], xr[:].bitcast(f32))
                            x_r[k] = xr
                        wr = wr_pool.tile([P, NF], f32r, tag="wr")
                        nc.sync.dma_start(
                            wr[:].bitcast(f32),
                            wT[k * P:(k + 1) * P, n * NF:(n + 1) * NF])
                        nc.vector.tensor_copy(wr[:], wr[:].bitcast(f32))
                        for m in range(MT):
                            nc.tensor.matmul(
                                psums[m][:],
                                x_r[k][:, m * P:(m + 1) * P],
                                wr[:],
                                start=(k == 0),
                                stop=(k == KT - 1),
                            )
                    for m in range(MT):
                        ob = ob_pool.tile([P, NF], f32, tag="ob")
                        nc.any.tensor_add(ob[:], psums[m][:], bias_t[:])
                        nc.scalar.dma_start(
                            out[m * P:(m + 1) * P, n * NF:(n + 1) * NF], ob[:]
                        )

    nc.compile()
    return nc


def _build_nc_wstat(repeats=None):
    """W-stationary layout: out.T[d_out, tok] per core; lhsT = W tile reused
    across 2 moving token-blocks (halves exposed f32r weight-load cost);
    bias is per-partition via tensor_scalar_add."""
    import contextlib

    import concourse.mybir as mybir
    import concourse.tile as tile
    from concourse import bacc

    DG = 8            # d_out groups of NF=512 (4 d-tiles of 128)
    TB = TOK_SH // NF  # 2 token blocks

    nc = bacc.Bacc(None, target_bir_lowering=False, debug=False)

    xT = nc.dram_tensor("xT", [D_IN, TOK_SH], mybir.dt.float32, kind="ExternalInput")
    wT = nc.dram_tensor("wT", [D_IN, D_OUT], mybir.dt.float32, kind="ExternalInput")
    bias_col = nc.dram_tensor(
        "bias_col", [P, D_OUT // P], mybir.dt.float32, kind="ExternalInput")
    outT = nc.dram_tensor(
        "outT", [D_OUT, TOK_SH], mybir.dt.float32, kind="ExternalOutput")

    with tile.TileContext(nc) as tc:
        with tc.tile_pool(name="xs", bufs=2) as xs_pool, \
             tc.tile_pool(name="xr", bufs=KT) as xr_pool, \
             tc.tile_pool(name="ws", bufs=4) as ws_pool, \
             tc.tile_pool(name="wr", bufs=6) as wr_pool, \
             tc.tile_pool(name="bias", bufs=1) as bias_pool, \
             tc.tile_pool(name="ob", bufs=4) as ob_pool, \
             tc.tile_pool(name="ps", bufs=8, space="PSUM") as ps_pool:

            if repeats is not None:
                loop_cm = tc.For_i(
                    0, repeats, 1,
                    hint_engines=(
                        mybir.EngineType.PE, mybir.EngineType.DVE,
                        mybir.EngineType.Activation, mybir.EngineType.SP,
                        mybir.EngineType.Pool,
                    ),
                )
            else:
                loop_cm = contextlib.nullcontext()

            with loop_cm:
                bias_t = bias_pool.tile([P, D_OUT // P], mybir.dt.float32, tag="bias")
                nc.sync.dma_start(bias_t[:], bias_col[:])

                x_r = [None] * KT

                for dg in range(DG):
                    psums = [
                        ps_pool.tile([P, NF], mybir.dt.float32, tag="ps", name="ps")
                        for _ in range(4 * TB)
                    ]
                    for k in range(KT):
                        if dg == 0:
                            xs = xs_pool.tile([P, TOK_SH], mybir.dt.float32, tag="xs")
                            nc.sync.dma_start(xs[:], xT[k * P:(k + 1) * P, :])
                            xr = xr_pool.tile([P, TOK_SH], mybir.dt.float32r, tag="xr")
                            nc.vector.tensor_copy(xr[:], xs[:])
                            x_r[k] = xr
                        ws = ws_pool.tile([P, NF], mybir.dt.float32, tag="ws")
                        nc.sync.dma_start(
                            ws[:], wT[k * P:(k + 1) * P, dg * NF:(dg + 1) * NF])
                        wr = wr_pool.tile([P, NF], mybir.dt.float32r, tag="wr")
                        nc.vector.tensor_copy(wr[:], ws[:])
                        for j in range(4):
                            for t in range(TB):
                                nc.tensor.matmul(
                                    psums[j * TB + t][:],
                                    wr[:, j * P:(j + 1) * P],
                                    x_r[k][:, t * NF:(t + 1) * NF],
                                    start=(k == 0),
                                    stop=(k == KT - 1),
                                )
                    for j in range(4):
                        d = dg * 4 + j
                        for t in range(TB):
                            ob = ob_pool.tile([P, NF], mybir.dt.float32, tag="ob")
                            nc.vector.tensor_scalar_add(
                                ob[:], psums[j * TB + t][:], bias_t[:, d:d + 1])
                            nc.sync.dma_start(
                                outT[d * P:(d + 1) * P, t * NF:(t + 1) * NF], ob[:])

    nc.compile()
    return nc


def _build_nc_wstat2(repeats=None):
    """W-stationary with half-group PSUM alternation: each d-group of 512
    outputs is processed as two halves of 4 PSUM tiles alternating between
    bank groups 0-3 and 4-7, so evictions of one half overlap compute of the
    next and the PE never waits on PSUM recycling.  W is read once as
    [128,256] half-tiles; x stays resident in f32r."""
    import contextlib

    import concourse.mybir as mybir
    import concourse.tile as tile
    from concourse import bacc

    DG = 8             # d_out groups of NF=512
    TB = TOK_SH // NF  # 2 token blocks
    HNF = NF // 2      # 256: W half-tile width

    nc = bacc.Bacc(None, target_bir_lowering=False, debug=False)

    xT = nc.dram_tensor("xT", [D_IN, TOK_SH], mybir.dt.float32, kind="ExternalInput")
    wT = nc.dram_tensor("wT", [D_IN, D_OUT], mybir.dt.float32, kind="ExternalInput")
    bias_col = nc.dram_tensor(
        "bias_col", [P, D_OUT // P], mybir.dt.float32, kind="ExternalInput")
    outT = nc.dram_tensor(
        "outT", [D_OUT, TOK_SH], mybir.dt.float32, kind="ExternalOutput")

    with tile.TileContext(nc) as tc:
        with tc.tile_pool(name="xs", bufs=2) as xs_pool, \
             tc.tile_pool(name="xr", bufs=KT) as xr_pool, \
             tc.tile_pool(name="ws", bufs=6) as ws_pool, \
             tc.tile_pool(name="wr", bufs=8) as wr_pool, \
             tc.tile_pool(name="bias", bufs=1) as bias_pool, \
             tc.tile_pool(name="ob", bufs=6) as ob_pool, \
             tc.tile_pool(name="ps", bufs=8, space="PSUM") as ps_pool:

            if repeats is not None:
                loop_cm = tc.For_i(
                    0, repeats, 1,
                    hint_engines=(
                        mybir.EngineType.PE, mybir.EngineType.DVE,
                        mybir.EngineType.Activation, mybir.EngineType.SP,
                        mybir.EngineType.Pool,
                    ),
                )
            else:
                loop_cm = contextlib.nullcontext()

            with loop_cm:
                bias_t = bias_pool.tile([P, D_OUT // P], mybir.dt.float32, tag="bias")
                nc.sync.dma_start(bias_t[:], bias_col[:])

                x_r = [None] * KT

                def evict(dg, h, psums):
                    # evictions of half (dg,h): d tiles dg*4+2h, dg*4+2h+1
                    evs = []
                    for j in range(2):
                        d = dg * 4 + 2 * h + j
                        for t in range(TB):
                            evs.append((d, t, psums[j * TB + t]))
                    return evs

                def emit_evict(ev):
                    d, t, psum = ev
                    ob = ob_pool.tile([P, NF], mybir.dt.float32, tag="ob", name="ob")
                    nc.vector.tensor_scalar_add(ob[:], psum[:], bias_t[:, d:d + 1])
                    nc.sync.dma_start(
                        outT[d * P:(d + 1) * P, t * NF:(t + 1) * NF], ob[:])

                pending = []
                for dg in range(DG):
                    for h in range(2):
                        psums = [
                            ps_pool.tile([P, NF], mybir.dt.float32, tag="ps",
                                         name="ps")
                            for _ in range(4)
                        ]
                        for k in range(KT):
                            if dg == 0 and h == 0:
                                xs = xs_pool.tile(
                                    [P, TOK_SH], mybir.dt.float32, tag="xs")
                                nc.sync.dma_start(xs[:], xT[k * P:(k + 1) * P, :])
                                xr = xr_pool.tile(
                                    [P, TOK_SH], mybir.dt.float32r, tag="xr")
                                nc.vector.tensor_copy(xr[:], xs[:])
                                x_r[k] = xr
                            ws = ws_pool.tile([P, HNF], mybir.dt.float32, tag="ws")
                            nc.sync.dma_start(
                                ws[:],
                                wT[k * P:(k + 1) * P,
                                   dg * NF + h * HNF:dg * NF + (h + 1) * HNF])
                            wr = wr_pool.tile([P, HNF], mybir.dt.float32r, tag="wr")
                            nc.vector.tensor_copy(wr[:], ws[:])
                            if pending and k < len(pending):
                                emit_evict(pending[k])
                            for j in range(2):
                                for t in range(TB):
                                    nc.tensor.matmul(
                                        psums[j * TB + t][:],
                                        wr[:, j * P:(j + 1) * P],
                                        x_r[k][:, t * NF:(t + 1) * NF],
                                        start=(k == 0),
                                        stop=(k == KT - 1),
                                    )
                        pending = evict(dg, h, psums)
                for ev in pending:
                    emit_evict(ev)

    nc.compile()
    return nc


def make_runner(nc, n_cores=N_CORES, replicated_inputs=()):
    """Build a reusable jitted SPMD callable for a compiled Bass module.

    Mirrors bass2jax.run_bass_via_pjrt's multi-core path, but returns the
    jitted function so repeated calls don't re-trace/re-compile.
    Inputs named in `replicated_inputs` use a replicated spec (pass the
    plain per-core array, no 8x concat)."""
    import jax
    import concourse.mybir as mybir
    from concourse import bass2jax
    from jax.experimental.shard_map import shard_map
    from jax.sharding import Mesh, PartitionSpec

    bass2jax.install_neuronx_cc_hook()

    partition_name = nc.partition_id_tensor.name if nc.partition_id_tensor else None
    in_names, out_names, out_avals, zero_outs = [], [], [], []
    for alloc in nc.m.functions[0].allocations:
        if not isinstance(alloc, mybir.MemoryLocationSet):
            continue
        name = alloc.memorylocations[0].name
        if alloc.kind == "ExternalInput":
            if name != partition_name:
                in_names.append(name)
        elif alloc.kind == "ExternalOutput":
            out_names.append(name)
            shape = tuple(alloc.tensor_shape)
            dtype = mybir.dt.np(alloc.dtype)
            out_avals.append(jax.core.ShapedArray(shape, dtype))
            zero_outs.append(np.zeros(shape, dtype))
    n_params = len(in_names)
    n_outs = len(out_avals)
    bind_in_names = list(in_names) + list(out_names)
    if partition_name is not None:
        bind_in_names.append(partition_name)

    def _body(*args):
        operands = list(args)
        if partition_name is not None:
            operands.append(bass2jax.partition_id_tensor())
        outs = bass2jax._bass_exec_p.bind(
            *operands,
            out_avals=tuple(out_avals),
            in_names=tuple(bind_in_names),
            out_names=tuple(out_names),
            lowering_input_output_aliases=(),
            sim_require_finite=True,
            sim_require_nnan=True,
            nc=nc,
        )
        return tuple(outs)

    devices = jax.devices()[:n_cores]
    mesh = Mesh(np.asarray(devices), ("core",))
    specs_map = {
        name: (PartitionSpec() if name in replicated_inputs
               else PartitionSpec("core"))
        for name in in_names
    }
    in_specs = tuple(specs_map[name] for name in in_names) + \
        (PartitionSpec("core"),) * n_outs
    out_specs = (PartitionSpec("core"),) * n_outs
    donate = tuple(range(n_params, n_params + n_outs))
    fn = jax.jit(
        shard_map(_body, mesh=mesh, in_specs=in_specs, out_specs=out_specs,
                  check_rep=False),
        donate_argnums=donate,
        keep_unused=True,
    )
    return {
        "fn": fn,
        "body": _body,
        "n_params": n_params,
        "in_names": in_names,
        "in_specs_map": specs_map,
        "out_names": out_names,
        "out_avals": out_avals,
        "zero_outs": zero_outs,
        "mesh": mesh,
        "n_cores": n_cores,
    }


LAYOUT = "xstat"  # "xstat" (out=[tok,d_out]) or "wstat" (out=[d_out,tok])


def build_nc(layout=None, repeats=None):
    layout = layout or LAYOUT
    builders = {"xstat": _build_nc, "xstat2": _build_nc_v2,
                "wstat": _build_nc_wstat, "wstat2": _build_nc_wstat2}
    return builders[layout](repeats=repeats)


def _get_runner(layout=None):
    layout = layout or LAYOUT
    key = f"runner_{layout}"
    if key not in _cache:
        repl = (("wT", "biasb") if layout in ("xstat", "xstat2")
                else ("wT", "bias_col"))
        _cache[key] = make_runner(build_nc(layout=layout), replicated_inputs=repl)
    return _cache[key]


def _circulant_expand(kernel):
    # W[p*b+i, q*b+j] = kernel[p, q, (i-j) % b]
    p, q, b = kernel.shape
    idx = (np.arange(b)[:, None] - np.arange(b)[None, :]) % b
    kc = kernel[:, :, idx]  # (p, q, b_i, b_j)
    return kc.transpose(0, 2, 1, 3).reshape(p * b, q * b)


def prep_inputs(x, base_weight, base_bias, c3a_kernel, layout=None):
    """Host-side prep: fold c3a into the weight; emit per-input concat arrays
    (axis 0 concat over cores, as shard_map expects)."""
    layout = layout or LAYOUT
    x = np.asarray(x, dtype=np.float32)
    base_weight = np.asarray(base_weight, dtype=np.float32)
    base_bias = np.asarray(base_bias, dtype=np.float32)
    c3a_kernel = np.asarray(c3a_kernel, dtype=np.float32)

    w_comb = base_weight + _circulant_expand(c3a_kernel) * (1.0 / D_IN)
    wT = np.ascontiguousarray(w_comb.T)                      # [D_IN, D_OUT]
    xT = np.ascontiguousarray(x.reshape(TOK, D_IN).T)        # [D_IN, TOK]

    # per-core shards, concatenated along axis 0 (shard_map splits axis 0)
    xT_cat = np.concatenate(
        [xT[:, c * TOK_SH:(c + 1) * TOK_SH] for c in range(N_CORES)], axis=0)
    if layout == "xstat":
        biasb = np.ascontiguousarray(
            np.broadcast_to(base_bias, (P, D_OUT)).astype(np.float32))
        return {"xT": xT_cat, "wT": wT, "biasb": biasb}
    else:
        bias_col = np.ascontiguousarray(base_bias.reshape(D_OUT // P, P).T)
        return {"xT": xT_cat, "wT": wT, "bias_col": bias_col}


def assemble_output(out_global, layout=None):
    """out_global: the concat-over-cores output array -> full (B,S,D_OUT)."""
    layout = layout or LAYOUT
    if layout == "xstat":
        # (N_CORES*TOK_SH, D_OUT), token-sharded in order
        return np.asarray(out_global).reshape(B, S, D_OUT)
    else:
        # (N_CORES*D_OUT, TOK_SH) -> [c, d, t] -> full [d, c*t]
        a = np.asarray(out_global).reshape(N_CORES, D_OUT, TOK_SH)
        full = a.transpose(1, 0, 2).reshape(D_OUT, TOK)
        return np.ascontiguousarray(full.T).reshape(B, S, D_OUT)


def kernel(x, base_weight, base_bias, c3a_kernel, **_):
    runner = _get_runner()
    cat = prep_inputs(x, base_weight, base_bias, c3a_kernel)
    ins = [cat[name] for name in runner["in_names"]]
    zeros = [
        np.zeros((N_CORES * z.shape[0], *z.shape[1:]), z.dtype)
        for z in runner["zero_outs"]
    ]
    out_arrs = runner["fn"](*ins, *zeros)
    return assemble_output(out_arrs[0])
